# revision 36
# baseline (speedup 1.0000x reference)
"""Trainium2 Bass kernel for CrossEfficientAttention (B=8, C=256, H=W=64, 4 heads).

Sharding: data-parallel over batch B — one sample per NeuronCore, no collectives.

Per-core math (sample x_s, c_s of shape [C, N], N = H*W = 4096):
    Q  = wq @ x_s                      (+ bq, folded into the exp's ACT bias)
    KV = wkv @ c_s                     (bkv[:C] cancels exactly in softmax over N;
                                        bkv[C:] handled as a rank-1 update of W)
    k  = softmax_N(K); q = softmax_head(Q * C**-0.25)
    context = k @ V^T ; out = wo @ (context @ q) + bo

Restructured for the PE array (out = lhsT.T @ rhs, contraction over partitions):
  * KV^T computed directly in [N, C] layout by using c_s tiles as lhsT.
  * k-softmax normalizer: ones-columns appended to V^T give row sums of exp(K)
    in column 256 of the context PSUM accumulator; context rows are then scaled
    by the reciprocal column (per-partition tensor_scalar) — no transposes.
  * wo folded in early: W^T = matmul(lhsT=context, rhs=wo^T) directly in [d, o]
    layout. The per-chunk output is then just out2 = W^T.T @ q.
  * q-softmax denominators: block-indicator matmul sums exp(Q) per head into a
    [4, 512] PSUM tile; 1/D = exp(-ln D) on ScalarE; broadcast back to 128
    partitions with a tiny selector matmul.

Hard-won scheduling facts (measured on hardware):
  * Per-instruction overheads on ACT/DVE are ~0.3-0.6us — splitting a
    [128,1024] op into halves to "shorten chains" costs more than it saves.
  * The PE p-state ramps 1.2->2.4 GHz only after ~4-5us of sustained PE
    activity, and steps back down after ~1-2us gaps. Warmup matmuls of any
    size only delay real work (each costs a full stationary load + stream);
    instead the start is made gap-free: per-KV-iteration input chunks keep
    the (still slow-clocked) PE fed from the first possible moment.
  * Inputs (x, cp, wkv, wq) ride in bf16 — halves the input stream for
    ~3.8e-3 rel err; all on-chip intermediates stay f32r (same PE speed).
  * The Q phase uses a DEEP skew (out runs 5 iterations behind q) so the
    exp->sum->ln->exp->broadcast->mul chain of the LAST chunk completes while
    the PE still has mid-phase outs to chew; the drain alternates two PSUM
    out-pools (reusing the dead q-pool's banks) and splits the drain copies
    across ACT+DVE so neither engine backlogs, and the clock never steps
    down mid-drain.
  * One HWDGE queue holds 6 in-flight DMAs; triggers cost ~650ns of SP time
    each. Input DMA runs well below peak for the first ~6us.
  * An SBUF->SBUF broadcast DMA (stride-0 source) for the 1/D broadcast
    crashed the device (NRT_EXEC_UNIT_UNRECOVERABLE) — keep the selector
    matmul.
"""

import numpy as np

import concourse.bass as bass
import concourse.tile as tile
from concourse import bacc, mybir
from concourse.bass import ts
from concourse.bass_utils import run_bass_kernel_spmd

B, C, H, W = 8, 256, 64, 64
N = H * W
NHEADS = 4
DHEAD = C // NHEADS
NCORES = 8
NSUPER = N // 256          # 16 double-n-tile iterations for the KV phase
NCHUNKS = N // 512         # 8 column chunks for the Q/output phase
SCALE = float(1.0 / np.sqrt(np.sqrt(np.float32(C))))
VW = C + 2                 # V^T tile row width (256 data + 2 ones cols; fp32r needs even free)
WPB = 2 * C + C              # bf16-able packed row width per c-half: wkvT|wqT
WPF = C + NHEADS             # f32r packed row width per c-half: woT|ind

F32 = mybir.dt.float32
F32R = mybir.dt.float32r
BF16 = mybir.dt.bfloat16
AF = mybir.ActivationFunctionType

_CACHE = {}


def _single_act_table():
    """Scope-patch the activation-table list so the table-load pass resolves
    both Exp and Ln to natural_log_exp_and_others (set ids stay positional,
    so only the function lists may change, not the order)."""
    import contextlib

    import concourse.bacc as cbacc
    from concourse.hw_specs import get_activation_tables

    @contextlib.contextmanager
    def scope():
        orig = cbacc.get_activation_tables

        def patched(arch):
            tabs = get_activation_tables(arch)
            return {
                k: (v if k == "natural_log_exp_and_others" else set())
                for k, v in tabs.items()
            }

        cbacc.get_activation_tables = patched
        try:
            yield
        finally:
            cbacc.get_activation_tables = orig

    return scope()


def _build(use_bq, use_bo, use_bv, mm_dtype):
    nc = bacc.Bacc("TRN2", target_bir_lowering=False, debug=False)
    # IDT: dtype of the DMA-heavy inputs (x, cp, wkv, wq) — bf16 halves the
    # input stream with one rounding ahead of the softmax averaging.
    # SDT: on-chip intermediates stay f32r (PE streams both at 1 col/cycle,
    # so bf16 there would only add rounding, not speed).
    IDT = mm_dtype
    SDT = F32R

    x = nc.dram_tensor("x", [C, N], IDT, kind="ExternalInput")
    cp = nc.dram_tensor("cp", [C, N], IDT, kind="ExternalInput")
    wpack = nc.dram_tensor("wpack", [128, 2 * WPB], IDT, kind="ExternalInput")
    wpackf = nc.dram_tensor("wpackf", [128, 2 * WPF], F32R, kind="ExternalInput")
    sel = nc.dram_tensor("sel", [NHEADS, C], F32R, kind="ExternalInput")
    if use_bq:
        bq_s = nc.dram_tensor("bq_s", [C, 1], F32, kind="ExternalInput")
    if use_bo:
        bo_c = nc.dram_tensor("bo_c", [C, 1], F32, kind="ExternalInput")
    if use_bv:
        bv_r = nc.dram_tensor("bv_r", [1, C], F32R, kind="ExternalInput")
        wosum = nc.dram_tensor("wosum", [1, C], F32R, kind="ExternalInput")
    y = nc.dram_tensor("y", [C, N], F32, kind="ExternalOutput")

    # DRAM views with the c-half dim split out so one DMA covers both halves
    cp2 = cp[:].rearrange("(u p) n -> p u n", u=2)
    x2 = x[:].rearrange("(u p) n -> p u n", u=2)
    y2 = y[:].rearrange("(u p) n -> p u n", u=2)

    with tile.TileContext(nc) as tc:
        with (
            tc.tile_pool(name="const", bufs=1) as cst,
            tc.tile_pool(name="big", bufs=1) as big,
            tc.tile_pool(name="eqp", bufs=5) as eqp,
            tc.tile_pool(name="qtp", bufs=4) as qtp,
            tc.tile_pool(name="o2p", bufs=3) as o2p,
            tc.tile_pool(name="dsb", bufs=3) as dsb,
        ):
            # --- packed weights; the KV-phase slice (wkvT) rides first ---
            wpk = cst.tile([128, 2 * WPB], IDT, name="wpk")
            wpk3 = wpk[:].rearrange("p (u w) -> p u w", u=2)
            wpack3 = wpack[:].rearrange("p (u w) -> p u w", u=2)
            wpkf = cst.tile([128, 2 * WPF], SDT, name="wpkf")
            wkvT_sb = [wpk[:, u * WPB : u * WPB + 2 * C] for u in range(2)]
            wqT_sb = [wpk[:, u * WPB + 2 * C : u * WPB + 3 * C] for u in range(2)]
            woT_sb = [wpkf[:, u * WPF : u * WPF + C] for u in range(2)]
            ind_sb = [wpkf[:, u * WPF + C : u * WPF + C + NHEADS] for u in range(2)]
            sel_sb = [cst.tile([NHEADS, 128], SDT, name=f"sel{u}") for u in range(2)]
            nc.scalar.dma_start(out=wpkf[:], in_=wpackf[:])
            for u in range(2):
                nc.scalar.dma_start(out=sel_sb[u][:], in_=sel[:, ts(u, 128)])
            if use_bq:
                bq_sb = [cst.tile([128, 1], F32, name=f"bq{u}") for u in range(2)]
                for u in range(2):
                    nc.scalar.dma_start(out=bq_sb[u][:], in_=bq_s[ts(u, 128), :])
            if use_bo:
                bo_sb = [cst.tile([128, 1], F32, name=f"bo{u}") for u in range(2)]
                for u in range(2):
                    nc.scalar.dma_start(out=bo_sb[u][:], in_=bo_c[ts(u, 128), :])
            if use_bv:
                bv_sb = cst.tile([1, C], SDT, name="bv_sb")
                nc.scalar.dma_start(out=bv_sb[:], in_=bv_r[:])
                wosum_sb = cst.tile([1, C], SDT, name="wosum_sb")
                nc.scalar.dma_start(out=wosum_sb[:], in_=wosum[:])

            # --- sample loads: graduated trigger sizes on the SP queue, both
            # c-halves per trigger. Early DMA bandwidth ramps slowly, so the
            # first KV iterations gate on small transfers; the bulk rides fat.
            cf_sb = big.tile([128, 2, N], IDT, name="cf_sb")
            xf_sb = big.tile([128, 2, N], IDT, name="xf_sb")
            nc.sync.dma_start(out=cf_sb[:, :, 0:256], in_=cp2[:, :, 0:256])
            nc.sync.dma_start(out=wpk3[:, 0, 0 : 2 * C], in_=wpack3[:, 0, 0 : 2 * C])
            nc.sync.dma_start(out=wpk3[:, 1, 0 : 2 * C], in_=wpack3[:, 1, 0 : 2 * C])
            # per-KV-iteration chunks while the PE is still at 1.2 GHz and DMA
            # below peak: each iteration gates on one small transfer, so the
            # PE never gaps (gaps reset the p-state ramp)
            nc.scalar.dma_start(out=cf_sb[:, :, 256:512], in_=cp2[:, :, 256:512])
            nc.scalar.dma_start(out=cf_sb[:, :, 512:768], in_=cp2[:, :, 512:768])
            for c0, c1 in ((768, 1024), (1024, 1280),
                           (1280, 1536), (1536, 2048), (2048, 2560)):
                nc.sync.dma_start(out=cf_sb[:, :, c0:c1], in_=cp2[:, :, c0:c1])
            nc.sync.dma_start(out=wpk3[:, :, 2 * C : WPB], in_=wpack3[:, :, 2 * C : WPB])
            nc.sync.dma_start(out=cf_sb[:, :, 2560:4096], in_=cp2[:, :, 2560:4096])
            nc.sync.dma_start(out=xf_sb[:, :, 0:2048], in_=x2[:, :, 0:2048])
            nc.sync.dma_start(out=xf_sb[:, :, 2048:4096], in_=x2[:, :, 2048:4096])

            # persistent W^T tiles (filled in the epilogue)
            WT_sb = [cst.tile([128, C], SDT, name=f"WT{u}") for u in range(2)]

            # manually-rotated V^T ring: ones columns pre-set once
            NVBUF = 4
            v2r = [cst.tile([128, 2 * VW], SDT, name=f"v2_{i}") for i in range(NVBUF)]
            # memset writes a 4-byte pattern: for f32r use an f32 bitcast view;
            # for bf16 write the packed pair pattern (two bf16 1.0s per word)
            ones_pair = float(
                np.frombuffer(np.uint32(0x3F803F80).tobytes(), np.float32)[0]
            )
            for i in range(NVBUF):
                for h in range(2):
                    o = v2r[i][:, h * VW + C : h * VW + C + 2].bitcast(F32)
                    nc.vector.memset(o, 1.0)

            eqs, psDs, rDs, psRbs, qts = {}, {}, {}, {}, {}

            def q_mms_into(j, psQ):
                for t in range(2):
                    for u in range(2):
                        nc.tensor.matmul(
                            psQ[:, t * 512 : (t + 1) * 512],
                            wqT_sb[u][:, ts(t, 128)],
                            xf_sb[:, u, ts(j, 512)],
                            start=(u == 0),
                            stop=(u == 1),
                        )

            def eq_act(j, psQ):
                eq = eqp.tile([128, 1024], SDT, name="eq", tag="eq")
                if use_bq:
                    for t in range(2):
                        nc.scalar.activation(
                            out=eq[:, t * 512 : (t + 1) * 512],
                            in_=psQ[:, t * 512 : (t + 1) * 512],
                            func=AF.Exp,
                            scale=SCALE,
                            bias=bq_sb[t][:],
                        )
                else:
                    nc.scalar.activation(
                        out=eq[:], in_=psQ[:], func=AF.Exp, scale=SCALE
                    )
                eqs[j] = eq

            # ============ KV phase: context = exp(K) @ [V^T | 1] ============
            # Software-pipelined by one iteration: the PE runs iteration i's
            # KV matmuls and iteration i-1's context matmuls back to back.
            with tc.tile_pool(name="psum_ctx", bufs=1, space="PSUM") as pctx:
                psCtx = [
                    pctx.tile([128, VW], F32, name=f"psCtx{u}") for u in range(2)
                ]
                with (
                    tc.tile_pool(name="psum_kv", bufs=3, space="PSUM") as pkv,
                    tc.tile_pool(name="kvsb", bufs=3) as kvsb,
                ):
                    eks = {}

                    def kv_mms(i):
                        psKV = pkv.tile([128, 1024], F32, name="psKV")
                        for h in range(2):
                            nt = 2 * i + h
                            for u in range(2):
                                nc.tensor.matmul(
                                    psKV[:, h * 512 : (h + 1) * 512],
                                    cf_sb[:, u, ts(nt, 128)],
                                    wkvT_sb[u],
                                    start=(u == 0),
                                    stop=(u == 1),
                                )
                        return psKV

                    def ctx_mms(i):
                        ek = eks.pop(i)
                        v2 = v2r[i % NVBUF]
                        for h in range(2):
                            for u in range(2):
                                nc.tensor.matmul(
                                    psCtx[u][:],
                                    ek[:, h, ts(u, 128)],
                                    v2[:, h * VW : (h + 1) * VW],
                                    start=(i == 0 and h == 0),
                                    stop=(i == NSUPER - 1 and h == 1),
                                    skip_group_check=True,
                                )

                    def kv_post(i, psKV):
                        ek = kvsb.tile([128, 2, C], SDT, name="ek")
                        nc.scalar.activation(
                            out=ek[:],
                            in_=psKV[:].rearrange("p (h c) -> p h c", h=2)[:, :, 0:C],
                            func=AF.Exp,
                        )
                        eks[i] = ek
                        v2 = v2r[i % NVBUF]
                        nc.vector.tensor_copy(
                            v2[:].rearrange("p (h w) -> p h w", h=2)[:, :, 0:C],
                            psKV[:].rearrange("p (h c) -> p h c", h=2)[:, :, C : 2 * C],
                        )

                    # the first two Q chunks ride inside the KV tail (their
                    # PSUM supertiles borrow the KV pool's slots) so their
                    # eq chains are done the moment the KV phase ends
                    for i in range(NSUPER):
                        psKV = kv_mms(i)
                        if i > 0:
                            ctx_mms(i - 1)
                        kv_post(i, psKV)
                        if i >= NSUPER - 2:
                            psQ = pkv.tile([128, 1024], F32, name="psKV", tag="psKV")
                            q_mms_into(i - (NSUPER - 2), psQ)
                            eq_act(i - (NSUPER - 2), psQ)
                    ctx_mms(NSUPER - 1)

                # ===== epilogue part 1 (DVE): normalize context rows =====
                rcol = [cst.tile([128, 1], F32, name=f"rcol{u}") for u in range(2)]
                ctx_sb = [cst.tile([128, C], SDT, name=f"ctx{u}") for u in range(2)]
                for u in range(2):
                    nc.vector.reciprocal(rcol[u][:], psCtx[u][:, C : C + 1])
                    nc.vector.tensor_scalar_mul(
                        out=ctx_sb[u][:], in0=psCtx[u][:, 0:C], scalar1=rcol[u][:]
                    )

            # ============ Q phase: out = W^T.T @ softmax_head(exp(Q*s)) ============
            # Supertile layout [128, 1024]: channel-half t at cols 512t.
            # DEEP skew: at iteration j the PE runs Q(j), D(j-2), Rb(j-3),
            # out(j-5); DVE runs mul(j-4) + the out copies. Every chunk's
            # softmax chain finishes ~2.5 iterations before its out matmuls,
            # so the drain after the last Q supertile is pure PE work.
            with (
                tc.tile_pool(name="psd", bufs=1, space="PSUM") as pd,
                tc.tile_pool(name="psrb", bufs=1, space="PSUM") as prb,
            ):
                def d_mms(j):
                    psD = pd.tile([NHEADS, 512], F32, name="psD")
                    for t in range(2):
                        nc.tensor.matmul(
                            psD[:],
                            ind_sb[t],
                            eqs[j][:, t * 512 : (t + 1) * 512],
                            start=(t == 0),
                            stop=(t == 1),
                        )
                    psDs[j] = psD

                def r_acts(j):
                    lnD = dsb.tile([NHEADS, 512], F32, name="lnD")
                    nc.scalar.activation(out=lnD[:], in_=psDs.pop(j)[:], func=AF.Ln)
                    rD = dsb.tile([NHEADS, 512], SDT, name="rD")
                    nc.scalar.activation(out=rD[:], in_=lnD[:], func=AF.Exp, scale=-1.0)
                    rDs[j] = rD

                def rb_mms(j):
                    psRb = prb.tile([128, 1024], F32, name="psRb")
                    rD = rDs.pop(j)
                    for t in range(2):
                        nc.tensor.matmul(
                            psRb[:, t * 512 : (t + 1) * 512],
                            sel_sb[t][:],
                            rD[:],
                            start=True,
                            stop=True,
                        )
                    psRbs[j] = psRb

                def q_mul(j):
                    qt = qtp.tile([128, 1024], SDT, name="qt", tag="qt")
                    nc.vector.tensor_mul(qt[:], eqs.pop(j)[:], psRbs.pop(j)[:])
                    qts[j] = qt

                # hoist chunks 0/1's D stages ahead of the W^T fold: the PE
                # chews them while the DVE runs the context-normalize chain
                d_mms(0)
                r_acts(0)
                d_mms(1)
                r_acts(1)

                # ===== epilogue part 2: fold wo, W^T = ctx.T @ woT =====
                with tc.tile_pool(name="psum_w", bufs=1, space="PSUM") as pw:
                    psW = [pw.tile([128, C], F32, name=f"psW{v}") for v in range(2)]
                    for v in range(2):
                        for u in range(2):
                            nc.tensor.matmul(
                                psW[v][:],
                                ctx_sb[u][:, ts(v, 128)],
                                woT_sb[u],
                                start=(u == 0),
                                stop=(u == 1) and not use_bv,
                                skip_group_check=True,
                            )
                        if use_bv:
                            # context gains +bv[d'] per row (sum_n k = 1), so
                            # W^T += bv (X) rowsum(wo): a K=1 rank-1 matmul.
                            nc.tensor.matmul(
                                psW[v][:],
                                bv_sb[:, ts(v, 128)],
                                wosum_sb[:],
                                start=False,
                                stop=True,
                                skip_group_check=True,
                            )
                        nc.vector.tensor_copy(WT_sb[v][:], psW[v][:])

                def out_store(pool, j):
                    psO = pool.tile([128, 1024], F32, name="psO", tag="psO")
                    o2 = o2p.tile([128, 2, 512], F32, name="o2", tag="o2")
                    qt = qts.pop(j)
                    for t in range(2):
                        for u in range(2):
                            nc.tensor.matmul(
                                psO[:, t * 512 : (t + 1) * 512],
                                WT_sb[u][:, ts(t, 128)],
                                qt[:, u * 512 : (u + 1) * 512],
                                start=(u == 0),
                                stop=(u == 1),
                            )
                    if use_bo:
                        for t in range(2):
                            nc.vector.tensor_scalar_add(
                                out=o2[:, t, :],
                                in0=psO[:, t * 512 : (t + 1) * 512],
                                scalar1=bo_sb[t][:],
                            )
                        nc.sync.dma_start(out=y2[:, :, ts(j, 512)], in_=o2[:])
                    elif j >= 3:
                        # drain chunks: the q supertiles are done, so ACT has
                        # slack — split the copy across ACT+DVE to halve the
                        # DVE backlog that otherwise gates the drain
                        nc.scalar.activation(
                            out=o2[:, 0, :], in_=psO[:, 0:512], func=AF.Copy
                        )
                        if j == NCHUNKS - 1:
                            nc.sync.dma_start(
                                out=y2[:, 0:1, ts(j, 512)], in_=o2[:, 0:1, :]
                            )
                        nc.vector.tensor_copy(o2[:, 1, :], psO[:, 512:1024])
                        if j == NCHUNKS - 1:
                            nc.sync.dma_start(
                                out=y2[:, 1:2, ts(j, 512)], in_=o2[:, 1:2, :]
                            )
                        else:
                            nc.sync.dma_start(out=y2[:, :, ts(j, 512)], in_=o2[:])
                    else:
                        nc.vector.tensor_copy(
                            o2[:], psO[:].rearrange("p (t c) -> p t c", t=2)
                        )
                        nc.sync.dma_start(out=y2[:, :, ts(j, 512)], in_=o2[:])

                with tc.tile_pool(name="pso", bufs=1, space="PSUM") as po:
                    with tc.tile_pool(name="psq", bufs=1, space="PSUM") as pq:
                        def q_mms(j):
                            psQ = pq.tile([128, 1024], F32, name="psQ")
                            q_mms_into(j, psQ)
                            return psQ

                        for j in range(2, NCHUNKS):
                            psQ = q_mms(j)
                            if 2 <= j - 2:
                                d_mms(j - 2)
                            if 0 <= j - 3:
                                rb_mms(j - 3)
                                q_mul(j - 3)
                            if 0 <= j - 5:
                                out_store(po, j - 5)
                            eq_act(j, psQ)
                            if 2 <= j - 2:
                                r_acts(j - 2)

                    # q supertiles done -> psq's 2 banks are free; drain the
                    # remaining outs alternating between two pools so out(c+1)
                    # never waits for out(c)'s PSUM->SBUF copy
                    with tc.tile_pool(name="pso2", bufs=1, space="PSUM") as po2:
                        for j in range(NCHUNKS, NCHUNKS + 6):
                            if j - 2 < NCHUNKS:
                                d_mms(j - 2)
                            if j - 3 < NCHUNKS:
                                rb_mms(j - 3)
                                q_mul(j - 3)
                            if j - 5 < NCHUNKS:
                                out_store(po2 if (j - 5) % 2 else po, j - 5)
                            if j - 2 < NCHUNKS:
                                r_acts(j - 2)

    nc.compile()
    return nc


def _get_nc(use_bq, use_bo, use_bv, mm_dtype):
    key = (use_bq, use_bo, use_bv, str(mm_dtype))
    if key not in _CACHE:
        with _single_act_table():
            _CACHE[key] = _build(use_bq, use_bo, use_bv, mm_dtype)
    return _CACHE[key]


def _to_mdt(a, mm_dtype):
    if mm_dtype == BF16:
        import ml_dtypes

        return np.ascontiguousarray(a.astype(ml_dtypes.bfloat16))
    return np.ascontiguousarray(a)


def kernel(x, cproj, wq, bq, wkv, bkv, wo, bo, _mm_dtype=BF16, _results_hook=None):
    x = np.ascontiguousarray(np.asarray(x, dtype=np.float32).reshape(B, C, N))
    cf = np.ascontiguousarray(np.asarray(cproj, dtype=np.float32).reshape(B, C, N))
    wq = np.asarray(wq, dtype=np.float32)
    wkv = np.asarray(wkv, dtype=np.float32)
    wo = np.asarray(wo, dtype=np.float32)
    bq = np.asarray(bq, dtype=np.float32)
    bkv = np.asarray(bkv, dtype=np.float32)
    bo = np.asarray(bo, dtype=np.float32)

    use_bq = bool(np.any(bq != 0))
    use_bo = bool(np.any(bo != 0))
    bv = bkv[C:]
    use_bv = bool(np.any(bv != 0))

    wqT = np.ascontiguousarray(wq.T)
    wkvT = np.ascontiguousarray(wkv.T)
    woT = np.ascontiguousarray(wo.T)
    ind = np.zeros((C, NHEADS), np.float32)
    ind[np.arange(C), np.arange(C) // DHEAD] = 1.0
    sel = np.ascontiguousarray(ind.T)

    # packed weights: bf16-able [wkvT | wqT] and f32 [woT | ind] per c-half
    wpack = np.zeros((128, 2 * WPB), np.float32)
    wpackf = np.zeros((128, 2 * WPF), np.float32)
    for u in range(2):
        r = slice(u * 128, (u + 1) * 128)
        wpack[:, u * WPB : u * WPB + 2 * C] = wkvT[r]
        wpack[:, u * WPB + 2 * C : u * WPB + 3 * C] = wqT[r]
        wpackf[:, u * WPF : u * WPF + C] = woT[r]
        wpackf[:, u * WPF + C : u * WPF + C + NHEADS] = ind[r]

    nc = _get_nc(use_bq, use_bo, use_bv, _mm_dtype)

    base = {
        "wpack": _to_mdt(wpack, _mm_dtype),
        "wpackf": wpackf,
        "sel": sel,
    }
    if use_bq:
        base["bq_s"] = (SCALE * bq).reshape(C, 1)
    if use_bo:
        base["bo_c"] = bo.reshape(C, 1)
    if use_bv:
        base["bv_r"] = bv.reshape(1, C)
        base["wosum"] = wo.sum(axis=1).reshape(1, C)

    in_maps = [
        dict(base, x=_to_mdt(x[b], _mm_dtype), cp=_to_mdt(cf[b], _mm_dtype))
        for b in range(B)
    ]
    res = run_bass_kernel_spmd(nc, in_maps, list(range(NCORES)))
    if _results_hook is not None:
        _results_hook(res)
    out = np.stack([res.results[b]["y"] for b in range(B)], axis=0)
    return out.reshape(B, C, H, W)
